# revision 17
# baseline (speedup 1.0000x reference)
"""Trainium2 Bass kernel for ClinicalStateFormationOperator (optimized).

Full-input contract: kernel(**inputs) takes the complete (unsharded) numpy
inputs and returns the full [B, T, V, D] output. Work is sharded across 8
NeuronCores as (batch, head-group): core c handles batch c//2 and heads
(c%2)*4 .. (c%2)*4+3. Each core computes its 4 heads' attention and the
partial output projection; the host sums the two partial projections per
batch and adds the output bias.

Structure (per core, N = T*V = 1536 tokens, head_dim = 64):

* Scores are computed transposed (keys on partitions, queries free) in ONE
  float32r K=128 matmul per [128k x 512q] tile by packing the contraction:
    rows   0:64   kT_h               |  qT_h          (content; 1/sqrt(64)
                                        folded into Wq host-side)
    rows  64:67   M'_h [foT;1]       |  [foT;1]       (observation)
    rows  67:99   [K%32==j] selector |  VB_h[Q%32,j]  (variable bias)
    rows  99:115  A_hj[s,K] values   |  [(Q//32)%16==s] (time bias; the 16
                  basis rows span q-chunk j's time bins, re-DMA'd per
                  (head, q-chunk))
  The observation scores have rank 2 (fo is [N,2]):
  oq.ok^T*os = [fo|1] M' [fo|1]^T with M' = [[Woq_h Wok_h^T, Woq_h bok],
  [boq Wok_h^T, boq.bok]]*os host-computed [3,3] per head - so there are
  no oq/ok projections on device at all.

* E^T = exp(scores^T) on the Act engine, written directly as bf16
  (attention is sharply peaked for outlier obs rows; bf16's 0.2% error
  passes the tolerance where fp8's 4-6% does not - measured).

* attn@v: [v_h | ones]^T @ E^T in bf16 (full PE rate); the 64 ones
  columns replicate the softmax denominator across partitions.
  OT^T = out^T * reciprocal(denom) on DVE.

* y_partial = OT^T_heads @ Wo_rows in float32r (host sums core pairs and
  adds bo). All projections run float32r for accuracy.

Engine budget (cost model): PE 81.9us (bound), Act 73.1us (48 exps),
DVE ~45us (all psum->sbuf copies + recip/mul), SP ~23us of DMA dispatch.
"""

import numpy as np
import ml_dtypes

import concourse.bass as bass
import concourse.mybir as mybir
import concourse.tile as tile
from concourse.bass_utils import run_bass_kernel_spmd

V = 32
T = 48
D = 512
H = 8
HD = D // H          # 64
OD = 16
B = 4
N = T * V            # 1536
HPC = 4              # heads per core
NCORES = 8
SCALE = 1.0 / np.sqrt(HD)
OBS_SCALE = 1.0 / np.sqrt(OD)

F32 = mybir.dt.float32
F32R = mybir.dt.float32r
BF16 = mybir.dt.bfloat16
EXP = mybir.ActivationFunctionType.Exp

KC = N // 128        # 12 key chunks of 128
QC = N // 512        # 3 query chunks of 512


def _split_waits(nc, max_waits=1):
    """Walrus in this container allows only one sync-wait slot per
    instruction; spill extra waits onto preceding same-engine NoOps."""
    def fix_bb(bb):
        changed = False
        new = []
        for inst in bb.instructions:
            si = inst.sync_info
            if si is not None and len(si.on_wait) > max_waits:
                waits = list(si.on_wait)
                for w in waits[:-max_waits]:
                    new.append(mybir.InstNoOp(
                        name=nc.get_next_instruction_name(),
                        engine=inst.engine, ins=[], outs=[],
                        sync_info=mybir.SyncInfo(on_wait=[w], on_update=[])))
                    changed = True
                si.on_wait = waits[-max_waits:]
            new.append(inst)
        if changed:
            bb.instructions = new
        for sub in getattr(bb, 'blocks', []) or []:
            fix_bb(sub)
    for f in nc.m.functions:
        for bb in f.blocks:
            fix_bb(bb)


def _build(with_bias=False):
    nc = bass.Bass()

    # ---- per-core DRAM I/O (data differs per core, program is SPMD) ----
    fhT = nc.dram_tensor('fhT', [D, N], F32R, kind='ExternalInput')
    wq = nc.dram_tensor('wq', [D, HPC * HD], F32R, kind='ExternalInput')
    wk = nc.dram_tensor('wk', [D, HPC * HD], F32R, kind='ExternalInput')
    wv = nc.dram_tensor('wv', [D, HPC * HD], F32R, kind='ExternalInput')
    wo = nc.dram_tensor('wo', [2, 128, D], F32R, kind='ExternalInput')
    # static pack rows (rows 64:128 of each head's packs), host-prepped
    qstat = nc.dram_tensor('qstat', [HPC, 64, N], F32R, kind='ExternalInput')
    kstat = nc.dram_tensor('kstat', [HPC, 64, N], F32R, kind='ExternalInput')
    # per-(head, q-chunk) time-bias value rows
    apack = nc.dram_tensor('apack', [HPC, QC, 16, N], F32R,
                           kind='ExternalInput')
    if with_bias:
        bqr = nc.dram_tensor('bqr', [1, HPC * HD], F32R, kind='ExternalInput')
        bkr = nc.dram_tensor('bkr', [1, HPC * HD], F32R, kind='ExternalInput')
        bvr = nc.dram_tensor('bvr', [1, HPC * HD], F32R, kind='ExternalInput')
        onesd = nc.dram_tensor('onesd', [1, 512], F32R, kind='ExternalInput')
    out = nc.dram_tensor('out', [N, D], F32, kind='ExternalOutput')

    with tile.TileContext(nc) as tc:
        with tc.tile_pool(name='cst', bufs=1) as cst, \
             tc.tile_pool(name='big', bufs=1) as big, \
             tc.tile_pool(name='work', bufs=3) as work, \
             tc.tile_pool(name='et', bufs=3) as etp, \
             tc.tile_pool(name='ps3', bufs=2, space='PSUM') as ps3, \
             tc.tile_pool(name='ps', bufs=2, space='PSUM') as ps:

            # ---- DMA order: wq + fhT chunks first so PE starts early ----
            t_wq = cst.tile([128, 4, HPC * HD], F32R)
            nc.sync.dma_start(t_wq[:], wq[:].rearrange('(o p) n -> p o n', p=128))
            t_fhT = big.tile([128, 4, N], F32R)
            fhT_r = fhT[:].rearrange('(o p) n -> p o n', p=128)
            for kk in range(4):
                nc.sync.dma_start(t_fhT[:, kk, :], fhT_r[:, kk, :])
            t_wk = cst.tile([128, 4, HPC * HD], F32R)
            nc.sync.dma_start(t_wk[:], wk[:].rearrange('(o p) n -> p o n', p=128))
            t_wv = cst.tile([128, 4, HPC * HD], F32R)
            nc.sync.dma_start(t_wv[:], wv[:].rearrange('(o p) n -> p o n', p=128))

            # score packs per head [128, N]; static rows 64:128 from tables
            t_qp = [big.tile([128, N], F32R, tag=f'qp{hh}', name=f'qp{hh}')
                    for hh in range(HPC)]
            t_kp = [big.tile([128, N], F32R, tag=f'kp{hh}', name=f'kp{hh}')
                    for hh in range(HPC)]
            for hh in range(HPC):
                nc.sync.dma_start(t_qp[hh][64:128, :], qstat[hh])
                nc.sync.dma_start(t_kp[hh][64:128, :], kstat[hh])
            t_wo = cst.tile([128, 2, D], F32R)
            nc.sync.dma_start(t_wo[:], wo[:].rearrange('o p n -> p o n'))
            if with_bias:
                t_bq = cst.tile([1, HPC * HD], F32R)
                nc.sync.dma_start(t_bq[:], bqr[:])
                t_bk = cst.tile([1, HPC * HD], F32R)
                nc.sync.dma_start(t_bk[:], bkr[:])
                t_bv = cst.tile([1, HPC * HD], F32R)
                nc.sync.dma_start(t_bv[:], bvr[:])
                t_ones = cst.tile([1, 512], F32R)
                nc.sync.dma_start(t_ones[:], onesd[:])

            # v in natural layout [tok, kc, head, v64|ones64], bf16
            t_v = big.tile([128, KC, HPC, 128], BF16, tag='vall', name='vall')
            nc.gpsimd.memset(t_v[:, :, :, 64:128], 1.0)
            # attention-out^T (pairs stacked: rows = 2x64 ch, dim1 = pair)
            t_ot = big.tile([128, 2, N], F32R, tag='ot', name='ot')

            # ---- stage 1: projections (f32r), copied into packs ----
            def emit_qk(m):
                for (w_t, b_name, pack) in ((t_wq, 'bq', t_qp),
                                            (t_wk, 'bk', t_kp)):
                    for j in range(QC):
                        p_qt = ps.tile([128, 512], F32, tag='mm', name='p_qt')
                        for kk in range(4):
                            nc.tensor.matmul(
                                p_qt[:], w_t[:, kk, m * 128:(m + 1) * 128],
                                t_fhT[:, kk, j * 512:(j + 1) * 512],
                                start=(kk == 0),
                                stop=(not with_bias and kk == 3))
                        if with_bias:
                            bt = t_bq if b_name == 'bq' else t_bk
                            nc.tensor.matmul(
                                p_qt[:], bt[:, m * 128:(m + 1) * 128],
                                t_ones[:], start=False, stop=True)
                        for s in range(2):
                            hh = 2 * m + s
                            nc.vector.tensor_copy(
                                pack[hh][0:64, j * 512:(j + 1) * 512],
                                p_qt[s * 64:(s + 1) * 64, :])

            emit_qk(0)
            # v: psum [128 tok, 2 kc, 256 ch] per 2-chunk batch
            for kc2 in range(KC // 2):
                p_v = ps.tile([128, 2, HPC * HD], F32, tag='mm', name='p_v')
                for sub in range(2):
                    kc = 2 * kc2 + sub
                    for kk in range(4):
                        nc.tensor.matmul(
                            p_v[:, sub, :],
                            t_fhT[:, kk, kc * 128:(kc + 1) * 128],
                            t_wv[:, kk, :], start=(kk == 0),
                            stop=(not with_bias and kk == 3))
                    if with_bias:
                        nc.tensor.matmul(p_v[:, sub, :], t_ones[:, 0:128],
                                         t_bv[:], start=False, stop=True)
                # one strided bf16 cast-copy for both chunks x 4 heads (DVE)
                nc.vector.tensor_copy(
                    t_v[:, 2 * kc2:2 * kc2 + 2, :, 0:64],
                    p_v[:].rearrange('p s (h c) -> p s h c', c=HD))
            emit_qk(1)

            # ---- stage 2: scores -> exp -> attn@v -> normalize, per (j, h) --
            for j in range(QC):
                for hh in range(HPC):
                    # time-bias value rows for this (head, q-chunk)
                    nc.sync.dma_start(t_kp[hh][99:115, :], apack[hh, j])
                    t_et = etp.tile([128, KC, 512], BF16, tag='et',
                                    name='t_et')
                    p_ot = ps.tile([128, 512], F32, tag='mm', name='p_ot')
                    for g in range(4):
                        p_s3 = ps3.tile([128, 3, 512], F32, tag='s3',
                                        name='p_s3')
                        for i3 in range(3):
                            kc = 3 * g + i3
                            nc.tensor.matmul(
                                p_s3[:, i3, :],
                                t_kp[hh][:, kc * 128:(kc + 1) * 128],
                                t_qp[hh][:, j * 512:(j + 1) * 512],
                                start=True, stop=True)
                        nc.scalar.activation(
                            t_et[:, 3 * g:3 * g + 3, :], p_s3[:], EXP)
                    for kc in range(KC):
                        nc.tensor.matmul(
                            p_ot[:], t_v[:, kc, hh, :], t_et[:, kc, :],
                            start=(kc == 0), stop=(kc == KC - 1))
                    # normalize: DVE recip + multiply
                    t_rec = work.tile([64, 512], F32, tag='rec', name='t_rec')
                    nc.vector.reciprocal(t_rec[:], p_ot[64:128, :])
                    nc.vector.tensor_mul(
                        t_ot[(hh % 2) * 64:(hh % 2) * 64 + 64, hh // 2,
                             j * 512:(j + 1) * 512],
                        p_ot[0:64, :], t_rec[:])
                # partial out-projection for this q-chunk's 4 row blocks
                for qq in range(4):
                    qc = 4 * j + qq
                    p_y = ps.tile([128, D], F32, tag='mm', name='p_y')
                    for p in range(2):
                        nc.tensor.matmul(p_y[:],
                                         t_ot[:, p, qc * 128:(qc + 1) * 128],
                                         t_wo[:, p, :], start=(p == 0),
                                         stop=(p == 1))
                    t_y = work.tile([128, D], F32, tag='y', name='t_y')
                    nc.vector.tensor_copy(t_y[:], p_y[:])
                    nc.sync.dma_start(out[qc * 128:(qc + 1) * 128, :], t_y[:])

    _split_waits(nc)
    return nc


_NC_CACHE = {}


def _get_nc(with_bias=False):
    if with_bias not in _NC_CACHE:
        _NC_CACHE[with_bias] = _build(with_bias)
    return _NC_CACHE[with_bias]


def _host_prep(h, observation_state, Wq, bq, Wk, bk, Wv, bv, Wo, bo,
               Woq, boq, Wok, bok, variable_bias, relative_time_bias,
               with_bias=False):
    f32 = np.float32
    h = np.asarray(h, f32)
    obs = np.asarray(observation_state, f32)
    Wq = np.asarray(Wq, f32)
    Wk = np.asarray(Wk, f32)
    Wv = np.asarray(Wv, f32)
    Wo = np.asarray(Wo, f32)
    Woq = np.asarray(Woq, f32)
    Wok = np.asarray(Wok, f32)
    boq = np.asarray(boq, f32)
    bok = np.asarray(bok, f32)
    vb = np.asarray(variable_bias, f32)
    rtb = np.asarray(relative_time_bias, f32)

    Kidx = np.arange(N)
    vK = Kidx % V                 # variable id of each token
    tK = Kidx // V                # time bin of each token
    var_sel = (vK[None, :] == np.arange(V)[:, None]).astype(f32)
    time_sel = ((tK[None, :] % 16) == np.arange(16)[:, None]).astype(f32)

    in_maps = []
    for c in range(NCORES):
        b, hg = divmod(c, 2)
        h0 = hg * HPC
        foT = np.ascontiguousarray(obs[b].reshape(N, 2).T)        # [2, N]
        fo1 = np.concatenate([foT, np.ones((1, N), f32)], axis=0)  # [3, N]

        qstat = np.zeros((HPC, 64, N), f32)
        kstat = np.zeros((HPC, 64, N), f32)
        ap = np.empty((HPC, QC, 16, N), f32)
        for hh in range(HPC):
            head = h0 + hh
            # rank-2 obs collapse: M' = [[M, Woq@bok],[Wok@boq, boq.bok]]*os
            Wq_h = Woq[:, head * OD:(head + 1) * OD]
            Wk_h = Wok[:, head * OD:(head + 1) * OD]
            bq_h = boq[head * OD:(head + 1) * OD]
            bk_h = bok[head * OD:(head + 1) * OD]
            Mp = np.zeros((3, 3), f32)
            Mp[0:2, 0:2] = Wq_h @ Wk_h.T
            Mp[0:2, 2] = Wq_h @ bk_h
            Mp[2, 0:2] = Wk_h @ bq_h
            Mp[2, 2] = bq_h @ bk_h
            Mp *= OBS_SCALE
            qstat[hh, 0:3] = fo1
            kstat[hh, 0:3] = Mp @ fo1
            # var bias: values on q side, selectors on k side
            qstat[hh, 3:35] = vb[head][vK, :].T
            kstat[hh, 3:35] = var_sel
            # time bias: selectors on q side, values on k side (per j)
            qstat[hh, 35:51] = time_sel
            for j in range(QC):
                idx = 16 * j + np.arange(16)[:, None] - tK[None, :] + (T - 1)
                ap[hh, j] = rtb[head][idx]
        cs, ce = h0 * HD, (h0 + HPC) * HD
        m = {
            'fhT': np.ascontiguousarray(h[b].reshape(N, D).T),
            'wq': np.ascontiguousarray(Wq[:, cs:ce] * SCALE),
            'wk': np.ascontiguousarray(Wk[:, cs:ce]),
            'wv': np.ascontiguousarray(Wv[:, cs:ce]),
            'wo': np.ascontiguousarray(Wo[cs:ce, :].reshape(2, 128, D)),
            'qstat': qstat,
            'kstat': kstat,
            'apack': ap,
        }
        if with_bias:
            bqv = np.asarray(bq, f32)[cs:ce] * SCALE
            m.update({
                'bqr': np.ascontiguousarray(bqv[None, :]),
                'bkr': np.ascontiguousarray(
                    np.asarray(bk, f32)[None, cs:ce]),
                'bvr': np.ascontiguousarray(
                    np.asarray(bv, f32)[None, cs:ce]),
                'onesd': np.ones((1, 512), f32),
            })
        in_maps.append(m)
    return in_maps


def kernel(**inputs):
    with_bias = any(
        np.any(np.asarray(inputs[k])) for k in ('bq', 'bk', 'bv'))
    nc = _get_nc(with_bias)
    in_maps = _host_prep(**inputs, with_bias=with_bias)
    res = run_bass_kernel_spmd(nc, in_maps, core_ids=list(range(NCORES)))
    bo = np.asarray(inputs['bo'], np.float32)
    outf = np.zeros((B, N, D), np.float32)
    for c in range(NCORES):
        outf[c // 2] += res.results[c]['out']
    outf += bo[None, None, :]
    return outf.reshape(B, T, V, D)


# revision 20
# speedup vs baseline: 1.0808x; 1.0808x over previous
"""Trainium2 Bass kernel for ClinicalStateFormationOperator (optimized).

Full-input contract: kernel(**inputs) takes the complete (unsharded) numpy
inputs and returns the full [B, T, V, D] output. Work is sharded across 8
NeuronCores as (batch, head-group): core c handles batch c//2 and heads
(c%2)*4 .. (c%2)*4+3. Each core computes its 4 heads' attention and the
partial output projection; the host sums the two partial projections per
batch and adds the output bias.

Structure (per core, N = T*V = 1536 tokens, head_dim = 64):

* Scores are computed transposed (keys on partitions, queries free) in ONE
  float32r K=128 matmul per [128k x 512q] tile by packing the contraction:
    rows   0:64   kT_h               |  qT_h          (content; 1/sqrt(64)
                                        folded into Wq host-side)
    rows  64:67   M'_h [foT;1]       |  [foT;1]       (observation)
    rows  67:99   [K%32==j] selector |  VB_h[Q%32,j]  (variable bias)
    rows  99:115  A_hj[s,K] values   |  [(Q//32)%16==s] (time bias; the 16
                  basis rows span q-chunk j's time bins, re-DMA'd per
                  (head, q-chunk))
  The observation scores have rank 2 (fo is [N,2]):
  oq.ok^T*os = [fo|1] M' [fo|1]^T with M' = [[Woq_h Wok_h^T, Woq_h bok],
  [boq Wok_h^T, boq.bok]]*os host-computed [3,3] per head - so there are
  no oq/ok projections on device at all.

* E^T = exp(scores^T) on the Act engine, written directly as bf16
  (attention is sharply peaked for outlier obs rows; bf16's 0.2% error
  passes the tolerance where fp8's 4-6% does not - measured).

* attn@v: [v_h | ones]^T @ E^T in bf16 (full PE rate); the 64 ones
  columns replicate the softmax denominator across partitions.
  OT^T = out^T * reciprocal(denom) on DVE.

* y_partial = OT^T_heads @ Wo_rows in float32r (host sums core pairs and
  adds bo). All projections run float32r for accuracy.

Engine budget (cost model): PE 81.9us (bound), Act 73.1us (48 exps),
DVE ~45us (all psum->sbuf copies + recip/mul), SP ~23us of DMA dispatch.
"""

import numpy as np
import ml_dtypes

import concourse.bass as bass
import concourse.mybir as mybir
import concourse.tile as tile
from concourse.bass_utils import run_bass_kernel_spmd

V = 32
T = 48
D = 512
H = 8
HD = D // H          # 64
OD = 16
B = 4
N = T * V            # 1536
HPC = 4              # heads per core
NCORES = 8
SCALE = 1.0 / np.sqrt(HD)
OBS_SCALE = 1.0 / np.sqrt(OD)

F32 = mybir.dt.float32
F32R = mybir.dt.float32r
BF16 = mybir.dt.bfloat16
EXP = mybir.ActivationFunctionType.Exp

KC = N // 128        # 12 key chunks of 128
QC = N // 512        # 3 query chunks of 512


def _split_waits(nc, max_waits=1):
    """Walrus in this container allows only one sync-wait slot per
    instruction; spill extra waits onto preceding same-engine NoOps."""
    def fix_bb(bb):
        changed = False
        new = []
        for inst in bb.instructions:
            si = inst.sync_info
            if si is not None and len(si.on_wait) > max_waits:
                waits = list(si.on_wait)
                for w in waits[:-max_waits]:
                    new.append(mybir.InstNoOp(
                        name=nc.get_next_instruction_name(),
                        engine=inst.engine, ins=[], outs=[],
                        sync_info=mybir.SyncInfo(on_wait=[w], on_update=[])))
                    changed = True
                si.on_wait = waits[-max_waits:]
            new.append(inst)
        if changed:
            bb.instructions = new
        for sub in getattr(bb, 'blocks', []) or []:
            fix_bb(sub)
    for f in nc.m.functions:
        for bb in f.blocks:
            fix_bb(bb)


def _build(with_bias=False):
    nc = bass.Bass()

    # ---- per-core DRAM I/O (data differs per core, program is SPMD) ----
    fhT = nc.dram_tensor('fhT', [D, N], F32R, kind='ExternalInput')
    wq = nc.dram_tensor('wq', [D, HPC * HD], F32R, kind='ExternalInput')
    wk = nc.dram_tensor('wk', [D, HPC * HD], F32R, kind='ExternalInput')
    wv = nc.dram_tensor('wv', [D, HPC * HD], F32R, kind='ExternalInput')
    wo = nc.dram_tensor('wo', [2, 128, D], F32R, kind='ExternalInput')
    # static pack rows (rows 64:128 of each head's packs), host-prepped
    qstat = nc.dram_tensor('qstat', [HPC, 64, N], F32R, kind='ExternalInput')
    kstat = nc.dram_tensor('kstat', [HPC, 64, N], F32R, kind='ExternalInput')
    # per-(head, q-chunk) time-bias value rows
    apack = nc.dram_tensor('apack', [HPC, QC, 16, N], F32R,
                           kind='ExternalInput')
    if with_bias:
        bqr = nc.dram_tensor('bqr', [1, HPC * HD], F32R, kind='ExternalInput')
        bkr = nc.dram_tensor('bkr', [1, HPC * HD], F32R, kind='ExternalInput')
        bvr = nc.dram_tensor('bvr', [1, HPC * HD], F32R, kind='ExternalInput')
        onesd = nc.dram_tensor('onesd', [1, 512], F32R, kind='ExternalInput')
    out = nc.dram_tensor('out', [N, D], F32, kind='ExternalOutput')

    with tile.TileContext(nc) as tc:
        with tc.tile_pool(name='cst', bufs=1) as cst, \
             tc.tile_pool(name='big', bufs=1) as big, \
             tc.tile_pool(name='work', bufs=3) as work, \
             tc.tile_pool(name='et', bufs=3) as etp, \
             tc.tile_pool(name='ps3', bufs=2, space='PSUM') as ps3, \
             tc.tile_pool(name='ps', bufs=2, space='PSUM') as ps:

            # ---- DMA order: wk + fhT chunks first (k-pack proj is the
            # critical path into stage 2), then statics, then the rest ----
            t_wk = cst.tile([128, 4, HPC * HD], F32R)
            nc.sync.dma_start(t_wk[:], wk[:].rearrange('(o p) n -> p o n', p=128))
            t_fhT = big.tile([128, 4, N], F32R)
            fhT_r = fhT[:].rearrange('(o p) n -> p o n', p=128)
            for kk in range(4):
                nc.sync.dma_start(t_fhT[:, kk, :], fhT_r[:, kk, :])
            t_wq = cst.tile([128, 4, HPC * HD], F32R)
            nc.sync.dma_start(t_wq[:], wq[:].rearrange('(o p) n -> p o n', p=128))
            t_wv = cst.tile([128, 4, HPC * HD], F32R)
            nc.sync.dma_start(t_wv[:], wv[:].rearrange('(o p) n -> p o n', p=128))

            # score packs per head [128, N]; static rows 64:128 from tables
            t_qp = [big.tile([128, N], F32R, tag=f'qp{hh}', name=f'qp{hh}')
                    for hh in range(HPC)]
            t_kp = [big.tile([128, N], F32R, tag=f'kp{hh}', name=f'kp{hh}')
                    for hh in range(HPC)]
            for hh in range(HPC):
                nc.sync.dma_start(t_kp[hh][64:128, :], kstat[hh])
                nc.sync.dma_start(t_qp[hh][64:128, :], qstat[hh])
                # first q-chunk's time-bias rows, prefetched at startup
                nc.sync.dma_start(t_kp[hh][99:115, :], apack[hh, 0])
            t_wo = cst.tile([128, 2, D], F32R)
            nc.sync.dma_start(t_wo[:], wo[:].rearrange('o p n -> p o n'))
            if with_bias:
                t_bq = cst.tile([1, HPC * HD], F32R)
                nc.sync.dma_start(t_bq[:], bqr[:])
                t_bk = cst.tile([1, HPC * HD], F32R)
                nc.sync.dma_start(t_bk[:], bkr[:])
                t_bv = cst.tile([1, HPC * HD], F32R)
                nc.sync.dma_start(t_bv[:], bvr[:])
                t_ones = cst.tile([1, 512], F32R)
                nc.sync.dma_start(t_ones[:], onesd[:])

            # v in natural layout [tok, kc, head, v64|ones64], bf16
            t_v = big.tile([128, KC, HPC, 128], BF16, tag='vall', name='vall')
            nc.gpsimd.memset(t_v[:, :, :, 64:128], 1.0)
            # attention-out^T (pairs stacked: rows = 2x64 ch, dim1 = pair)
            t_ot = big.tile([128, 2, N], F32R, tag='ot', name='ot')

            # ---- emission helpers ----
            def emit_proj(w_t, b_t, pack, m, j):
                """q or k projection for head-pair m, q-chunk j, both heads."""
                p_qt = ps.tile([128, 512], F32, tag='mm', name='p_qt')
                for kk in range(4):
                    nc.tensor.matmul(
                        p_qt[:], w_t[:, kk, m * 128:(m + 1) * 128],
                        t_fhT[:, kk, j * 512:(j + 1) * 512],
                        start=(kk == 0), stop=(not with_bias and kk == 3))
                if with_bias:
                    nc.tensor.matmul(p_qt[:], b_t[:, m * 128:(m + 1) * 128],
                                     t_ones[:], start=False, stop=True)
                for s in range(2):
                    hh = 2 * m + s
                    nc.vector.tensor_copy(
                        pack[hh][0:64, j * 512:(j + 1) * 512],
                        p_qt[s * 64:(s + 1) * 64, :])

            def emit_v(kc2):
                """v projection, 2 key chunks batched into one psum bank."""
                p_v = ps.tile([128, 2, HPC * HD], F32, tag='mm', name='p_v')
                for sub in range(2):
                    kc = 2 * kc2 + sub
                    for kk in range(4):
                        nc.tensor.matmul(
                            p_v[:, sub, :],
                            t_fhT[:, kk, kc * 128:(kc + 1) * 128],
                            t_wv[:, kk, :], start=(kk == 0),
                            stop=(not with_bias and kk == 3))
                    if with_bias:
                        nc.tensor.matmul(p_v[:, sub, :], t_ones[:, 0:128],
                                         t_bv[:], start=False, stop=True)
                nc.vector.tensor_copy(
                    t_v[:, 2 * kc2:2 * kc2 + 2, :, 0:64],
                    p_v[:].rearrange('p s (h c) -> p s h c', c=HD))

            ets = {}

            def emit_sc(j, hh):
                """scores + exp for (q-chunk j, head hh) -> bf16 E tile."""
                t_et = etp.tile([128, KC, 512], BF16, tag='et', name='t_et')
                ets[(j, hh)] = t_et
                for g in range(4):
                    p_s3 = ps3.tile([128, 3, 512], F32, tag='s3', name='p_s3')
                    for i3 in range(3):
                        kc = 3 * g + i3
                        nc.tensor.matmul(
                            p_s3[:, i3, :],
                            t_kp[hh][:, kc * 128:(kc + 1) * 128],
                            t_qp[hh][:, j * 512:(j + 1) * 512],
                            start=True, stop=True)
                    nc.scalar.activation(
                        t_et[:, 3 * g:3 * g + 3, :], p_s3[:], EXP)
                # prefetch next q-chunk's time-bias rows (WAR: after the
                # score matmuls above, giving the DMA a long window)
                if j + 1 < QC:
                    nc.sync.dma_start(t_kp[hh][99:115, :], apack[hh, j + 1])

            def emit_av(j, hh):
                """attn@v + normalization for (j, hh)."""
                t_et = ets.pop((j, hh))
                p_ot = ps.tile([128, 512], F32, tag='mm', name='p_ot')
                for kc in range(KC):
                    nc.tensor.matmul(
                        p_ot[:], t_v[:, kc, hh, :], t_et[:, kc, :],
                        start=(kc == 0), stop=(kc == KC - 1))
                t_rec = work.tile([64, 512], F32, tag='rec', name='t_rec')
                nc.vector.reciprocal(t_rec[:], p_ot[64:128, :])
                nc.vector.tensor_mul(
                    t_ot[(hh % 2) * 64:(hh % 2) * 64 + 64, hh // 2,
                         j * 512:(j + 1) * 512],
                    p_ot[0:64, :], t_rec[:])

            def emit_yp(j):
                """partial out-projection for q-chunk j's 4 row blocks."""
                for qq in range(4):
                    qc = 4 * j + qq
                    p_y = ps.tile([128, D], F32, tag='mm', name='p_y')
                    for p in range(2):
                        nc.tensor.matmul(p_y[:],
                                         t_ot[:, p, qc * 128:(qc + 1) * 128],
                                         t_wo[:, p, :], start=(p == 0),
                                         stop=(p == 1))
                    t_y = work.tile([128, D], F32, tag='y', name='t_y')
                    nc.vector.tensor_copy(t_y[:], p_y[:])
                    nc.sync.dma_start(out[qc * 128:(qc + 1) * 128, :], t_y[:])

            # ---- interleaved schedule: get the Act exp chain started as
            # early as possible (it is the serial floor), fill PE waits with
            # the remaining stage-1 projections, and run attn@v one head
            # behind the score/exp pipeline ----
            emit_proj(t_wk, t_bk if with_bias else None, t_kp, 0, 0)
            emit_proj(t_wk, t_bk if with_bias else None, t_kp, 0, 1)
            emit_proj(t_wk, t_bk if with_bias else None, t_kp, 0, 2)
            emit_proj(t_wq, t_bq if with_bias else None, t_qp, 0, 0)
            emit_sc(0, 0)                      # exp pipeline starts here
            for kc2 in range(KC // 2):
                emit_v(kc2)
            emit_sc(0, 1)
            emit_av(0, 0)
            emit_proj(t_wk, t_bk if with_bias else None, t_kp, 1, 0)
            emit_proj(t_wk, t_bk if with_bias else None, t_kp, 1, 1)
            emit_proj(t_wk, t_bk if with_bias else None, t_kp, 1, 2)
            emit_proj(t_wq, t_bq if with_bias else None, t_qp, 1, 0)
            emit_sc(0, 2)
            emit_av(0, 1)
            emit_proj(t_wq, t_bq if with_bias else None, t_qp, 0, 1)
            emit_sc(0, 3)
            emit_av(0, 2)
            emit_proj(t_wq, t_bq if with_bias else None, t_qp, 0, 2)
            emit_proj(t_wq, t_bq if with_bias else None, t_qp, 1, 1)
            emit_proj(t_wq, t_bq if with_bias else None, t_qp, 1, 2)
            emit_av(0, 3)
            emit_yp(0)
            for j in range(1, QC):
                emit_sc(j, 0)
                emit_sc(j, 1)
                emit_av(j, 0)
                emit_sc(j, 2)
                emit_av(j, 1)
                emit_sc(j, 3)
                emit_av(j, 2)
                emit_av(j, 3)
                emit_yp(j)

    _split_waits(nc)
    return nc


_NC_CACHE = {}


def _get_nc(with_bias=False):
    if with_bias not in _NC_CACHE:
        _NC_CACHE[with_bias] = _build(with_bias)
    return _NC_CACHE[with_bias]


def _host_prep(h, observation_state, Wq, bq, Wk, bk, Wv, bv, Wo, bo,
               Woq, boq, Wok, bok, variable_bias, relative_time_bias,
               with_bias=False):
    f32 = np.float32
    h = np.asarray(h, f32)
    obs = np.asarray(observation_state, f32)
    Wq = np.asarray(Wq, f32)
    Wk = np.asarray(Wk, f32)
    Wv = np.asarray(Wv, f32)
    Wo = np.asarray(Wo, f32)
    Woq = np.asarray(Woq, f32)
    Wok = np.asarray(Wok, f32)
    boq = np.asarray(boq, f32)
    bok = np.asarray(bok, f32)
    vb = np.asarray(variable_bias, f32)
    rtb = np.asarray(relative_time_bias, f32)

    Kidx = np.arange(N)
    vK = Kidx % V                 # variable id of each token
    tK = Kidx // V                # time bin of each token
    var_sel = (vK[None, :] == np.arange(V)[:, None]).astype(f32)
    time_sel = ((tK[None, :] % 16) == np.arange(16)[:, None]).astype(f32)

    in_maps = []
    for c in range(NCORES):
        b, hg = divmod(c, 2)
        h0 = hg * HPC
        foT = np.ascontiguousarray(obs[b].reshape(N, 2).T)        # [2, N]
        fo1 = np.concatenate([foT, np.ones((1, N), f32)], axis=0)  # [3, N]

        qstat = np.zeros((HPC, 64, N), f32)
        kstat = np.zeros((HPC, 64, N), f32)
        ap = np.empty((HPC, QC, 16, N), f32)
        for hh in range(HPC):
            head = h0 + hh
            # rank-2 obs collapse: M' = [[M, Woq@bok],[Wok@boq, boq.bok]]*os
            Wq_h = Woq[:, head * OD:(head + 1) * OD]
            Wk_h = Wok[:, head * OD:(head + 1) * OD]
            bq_h = boq[head * OD:(head + 1) * OD]
            bk_h = bok[head * OD:(head + 1) * OD]
            Mp = np.zeros((3, 3), f32)
            Mp[0:2, 0:2] = Wq_h @ Wk_h.T
            Mp[0:2, 2] = Wq_h @ bk_h
            Mp[2, 0:2] = Wk_h @ bq_h
            Mp[2, 2] = bq_h @ bk_h
            Mp *= OBS_SCALE
            qstat[hh, 0:3] = fo1
            kstat[hh, 0:3] = Mp @ fo1
            # var bias: values on q side, selectors on k side
            qstat[hh, 3:35] = vb[head][vK, :].T
            kstat[hh, 3:35] = var_sel
            # time bias: selectors on q side, values on k side (per j)
            qstat[hh, 35:51] = time_sel
            for j in range(QC):
                idx = 16 * j + np.arange(16)[:, None] - tK[None, :] + (T - 1)
                ap[hh, j] = rtb[head][idx]
        cs, ce = h0 * HD, (h0 + HPC) * HD
        m = {
            'fhT': np.ascontiguousarray(h[b].reshape(N, D).T),
            'wq': np.ascontiguousarray(Wq[:, cs:ce] * SCALE),
            'wk': np.ascontiguousarray(Wk[:, cs:ce]),
            'wv': np.ascontiguousarray(Wv[:, cs:ce]),
            'wo': np.ascontiguousarray(Wo[cs:ce, :].reshape(2, 128, D)),
            'qstat': qstat,
            'kstat': kstat,
            'apack': ap,
        }
        if with_bias:
            bqv = np.asarray(bq, f32)[cs:ce] * SCALE
            m.update({
                'bqr': np.ascontiguousarray(bqv[None, :]),
                'bkr': np.ascontiguousarray(
                    np.asarray(bk, f32)[None, cs:ce]),
                'bvr': np.ascontiguousarray(
                    np.asarray(bv, f32)[None, cs:ce]),
                'onesd': np.ones((1, 512), f32),
            })
        in_maps.append(m)
    return in_maps


def kernel(**inputs):
    with_bias = any(
        np.any(np.asarray(inputs[k])) for k in ('bq', 'bk', 'bv'))
    nc = _get_nc(with_bias)
    in_maps = _host_prep(**inputs, with_bias=with_bias)
    res = run_bass_kernel_spmd(nc, in_maps, core_ids=list(range(NCORES)))
    bo = np.asarray(inputs['bo'], np.float32)
    outf = np.zeros((B, N, D), np.float32)
    for c in range(NCORES):
        outf[c // 2] += res.results[c]['out']
    outf += bo[None, None, :]
    return outf.reshape(B, T, V, D)
